# revision 11
# baseline (speedup 1.0000x reference)
"""Multi-head attention (B=4, S=2048, D=2048, H=16) on 8 trn2 NeuronCores.

Sharding: tensor-parallel over heads — 2 heads per core. Each core computes
its heads' Q/K/V projections, full attention for those heads, and a partial
output projection (its 256 rows of wo). The host sums the 8 partial outputs.

v2 (fp16 datapath, gap-free PE schedule):
  - every matmul operand is fp16 (x, weights, Q/K/V, exp tiles, attention
    output); PSUM accumulation stays fp32. fp16 keeps 10 mantissa bits
    (same relative precision as tf32) while halving DMA/SBUF/LDWEIGHTS.
  - softmax denominator: exp tiles are partial-summed on the DVE (fp16
    tensor_tensor, depth-2 tree -> two [128, 2*QS] accumulators per head/
    span), then 4 ones-matmuls partition-reduce into PSUM. Removes the
    512 N=512 denominator matmuls (~119us of PE) for ~8us of DVE per span.
  - the ACT exp stream (1106ns per pair-tile) is slightly slower than the
    scores+AV matmuls (864ns), so the PE would drift into exp-waits. The
    out-projection of span qs-1 is therefore emitted as 16 small groups
    interleaved INTO span qs's pair loops (and the last span's into the
    next batch's Q projection pass) — the PE always has dependency-free
    work queued and batch boundaries have no drain/pstate-ramp gap.
  - PSUM->SBUF output copies are split 4/12 between ACT and DVE to keep
    both helper engines below the PE rate.
  - b=0 startup: wq chunk-DMAs interleave with the first x span, then all
    x spans stream before wk/wv (Q pass is DMA-paced, K/V passes are not);
    first matmul at ~13us.
"""
import os
import sys

sys.path.insert(0, "/opt/trn_rl_repo")
import numpy as np

B, S, D, H = 4, 2048, 2048, 16
HD = 128
NCORES = 8
HP = H // NCORES          # heads per core = 2
DC = HP * HD              # per-core slice of D = 256
TOK = B * S               # 8192
SCALE = HD ** -0.5
NDC = D // 128            # 16 contraction chunks for the projections
SPAN = 256                # token span per projection step
NSPAN = S // SPAN         # 8 spans per batch
QS = 512                  # query span in attention
NQS = S // QS             # 4
NKC = S // 128            # 16 key chunks
NPAIR = NKC // 2          # 8 key-chunk pairs

LAST_EXEC_NS = None
_BUILT = None


def _build():
    global _BUILT
    if _BUILT is not None:
        return _BUILT
    import concourse.tile as tile
    from concourse import bacc, mybir

    F16 = mybir.dt.float16
    F32 = mybir.dt.float32
    Exp = mybir.ActivationFunctionType.Exp
    Ident = mybir.ActivationFunctionType.Identity

    nc = bacc.Bacc("TRN2", target_bir_lowering=False, debug=False)
    xt = nc.dram_tensor("xt", [D, TOK], F16, kind="ExternalInput")
    wq = nc.dram_tensor("wq", [D, DC], F16, kind="ExternalInput")
    wk = nc.dram_tensor("wk", [D, DC], F16, kind="ExternalInput")
    wv = nc.dram_tensor("wv", [D, DC], F16, kind="ExternalInput")
    wo = nc.dram_tensor("wo", [DC, D], F16, kind="ExternalInput")
    bq2 = nc.dram_tensor("bq2", [HD, HP], F32, kind="ExternalInput")
    bk2 = nc.dram_tensor("bk2", [HD, HP], F32, kind="ExternalInput")
    ones = nc.dram_tensor("ones", [128, 128], F16, kind="ExternalInput")
    out = nc.dram_tensor("out", [TOK, D], F16, kind="ExternalOutput")

    with tile.TileContext(nc) as tc:
        with tc.tile_pool(name="const", bufs=1) as cpool, \
             tc.tile_pool(name="xp", bufs=8) as xpool, \
             tc.tile_pool(name="bt", bufs=1) as bpool, \
             tc.tile_pool(name="qk", bufs=2) as qkpool, \
             tc.tile_pool(name="avp", bufs=2) as avpool, \
             tc.tile_pool(name="pp", bufs=5) as ppool, \
             tc.tile_pool(name="s2", bufs=8) as s2pool, \
             tc.tile_pool(name="rc", bufs=2) as rpool, \
             tc.tile_pool(name="ot", bufs=2) as opool, \
             tc.tile_pool(name="ps", bufs=1, space="PSUM") as ps:

            wq_sb = cpool.tile([128, NDC, DC], F16)
            wk_sb = cpool.tile([128, NDC, DC], F16)
            wv_sb = cpool.tile([128, NDC, DC], F16)
            wo_sb = cpool.tile([128, HP, D], F16)
            ones_sb = cpool.tile([128, 128], F16)
            bq_sb = cpool.tile([HD, HP], F32)
            bk_sb = cpool.tile([HD, HP], F32)

            wq_r = wq.rearrange("(c p) n -> p c n", p=128)
            wk_r = wk.rearrange("(c p) n -> p c n", p=128)
            wv_r = wv.rearrange("(c p) n -> p c n", p=128)
            wo_r = wo.rearrange("(c p) n -> p c n", p=128)
            xt_r = xt.rearrange("(c p) t -> p c t", p=128)

            nc.sync.dma_start(out=bq_sb, in_=bq2[:, :])
            nc.sync.dma_start(out=bk_sb, in_=bk2[:, :])

            xts = {}

            def x_dma(b, sp):
                t0 = b * S + sp * SPAN
                xtl = xpool.tile([128, NDC, SPAN], F16, name=f"x{b}_{sp}",
                                 tag="x")
                xts[(b, sp)] = xtl
                nc.sync.dma_start(out=xtl, in_=xt_r[:, :, t0:t0 + SPAN])

            # --- b=0 startup: wq chunks interleave with the first x span;
            # remaining x spans stream before wk/wv (the Q pass is DMA-paced,
            # the K/V passes run much later) ---
            x00 = xpool.tile([128, NDC, SPAN], F16, name="x0_0", tag="x")
            xts[(0, 0)] = x00
            for i in range(4):
                nc.sync.dma_start(out=wq_sb[:, 4 * i:4 * i + 4, :],
                                  in_=wq_r[:, 4 * i:4 * i + 4, :])
                nc.sync.dma_start(out=x00[:, 4 * i:4 * i + 4, :],
                                  in_=xt_r[:, 4 * i:4 * i + 4, 0:SPAN])
            for sp in range(1, NSPAN):
                x_dma(0, sp)
            for i in range(2):
                nc.sync.dma_start(out=wk_sb[:, 8 * i:8 * i + 8, :],
                                  in_=wk_r[:, 8 * i:8 * i + 8, :])
            for i in range(2):
                nc.sync.dma_start(out=wv_sb[:, 8 * i:8 * i + 8, :],
                                  in_=wv_r[:, 8 * i:8 * i + 8, :])

            def pull(filler):
                if filler is not None:
                    next(filler, None)

            def proj_pass(b, w_sb, b_sb, dst):
                for sp in range(NSPAN):
                    xtl = xts[(b, sp)]
                    for h in range(HP):
                        pps = ps.tile([128, SPAN], F32, name="pps", tag="pj",
                                      bufs=2)
                        for c in range(NDC):
                            nc.tensor.matmul(
                                pps, w_sb[:, c, h * HD:(h + 1) * HD],
                                xtl[:, c, :], start=(c == 0),
                                stop=(c == NDC - 1))
                        nc.scalar.activation(
                            dst[:, h, sp * SPAN:(sp + 1) * SPAN], pps, Ident,
                            bias=b_sb[:, h:h + 1])

            def v_pass(b, v_b):
                for sp in range(NSPAN):
                    xtl = xts[(b, sp)]
                    vps = ps.tile([128, 2 * DC], F32, name="vps", tag="pj",
                                  bufs=2)
                    for tch in range(2):
                        for c in range(NDC):
                            nc.tensor.matmul(
                                vps[:, tch * DC:(tch + 1) * DC],
                                xtl[:, c, tch * 128:(tch + 1) * 128],
                                wv_sb[:, c, :], start=(c == 0),
                                stop=(c == NDC - 1))
                    for tch in range(2):
                        nc.scalar.copy(v_b[:, sp * 2 + tch, :],
                                       vps[:, tch * DC:(tch + 1) * DC])

            def warm_pair(qs, h, kp, qt_b, kt_b):
                # scores+exp of an upcoming half-span pair, emitted before
                # the current tail so the ACT exp stream never restarts cold
                q_sl = qt_b[:, h, qs * QS:(qs + 1) * QS]
                s_ps = ps.tile([128, 2 * QS], F32, name="s_ps", tag="s",
                               bufs=2)
                for j in range(2):
                    kc = 2 * kp + j
                    nc.tensor.matmul(
                        s_ps[:, j * QS:(j + 1) * QS],
                        kt_b[:, h, kc * 128:(kc + 1) * 128], q_sl,
                        start=True, stop=True)
                pt = ppool.tile([128, 2 * QS], F16, name="p_sb", tag="p")
                nc.scalar.activation(pt, s_ps, Exp, scale=SCALE)
                return pt

            def attn_span(qs, h, qt_b, kt_b, v_b, avt_b, filler=None,
                          warm=None):
                q_sl = qt_b[:, h, qs * QS:(qs + 1) * QS]
                av_ps = ps.tile([HD, QS], F32, name="av_ps", tag="acc",
                                bufs=2)
                p_tiles = []
                t_tiles = []
                dn_ps = None

                def emit_av(kp):
                    pt = p_tiles[kp]
                    for j in range(2):
                        kc = 2 * kp + j
                        nc.tensor.matmul(
                            av_ps, v_b[:, kc, h * HD:(h + 1) * HD],
                            pt[:, j * QS:(j + 1) * QS], start=(kc == 0),
                            stop=(kc == NKC - 1))

                def dn_mm(src, first=False, last=False):
                    nc.tensor.matmul(dn_ps, ones_sb, src[:, 0:QS],
                                     start=first, stop=False)
                    nc.tensor.matmul(dn_ps, ones_sb, src[:, QS:2 * QS],
                                     start=False, stop=last)

                for kp in range(NPAIR):
                    if warm is not None and kp < len(warm):
                        p_tiles.append(warm[kp])
                    else:
                        s_ps = ps.tile([128, 2 * QS], F32, name="s_ps",
                                       tag="s", bufs=2)
                        for j in range(2):
                            kc = 2 * kp + j
                            nc.tensor.matmul(
                                s_ps[:, j * QS:(j + 1) * QS],
                                kt_b[:, h, kc * 128:(kc + 1) * 128], q_sl,
                                start=True, stop=True)
                        pt = ppool.tile([128, 2 * QS], F16, name="p_sb",
                                        tag="p")
                        nc.scalar.activation(pt, s_ps, Exp, scale=SCALE)
                        p_tiles.append(pt)
                    # pair-sum tree: all four pair-adds on the idle GPSIMD
                    # (latency-tolerant), combine levels U/X on the DVE; dn
                    # ends up a single ones-matmul pair on the grand total
                    if kp in (1, 3, 5):
                        tt = s2pool.tile([128, 2 * QS], F16, name="t_sb",
                                         tag="s2")
                        nc.gpsimd.tensor_add(tt, p_tiles[kp - 1],
                                             p_tiles[kp])
                        t_tiles.append(tt)
                        if kp == 3:
                            ut = s2pool.tile([128, 2 * QS], F16, name="u_sb",
                                             tag="s2")
                            nc.vector.tensor_add(ut, t_tiles[0], t_tiles[1])
                            t_tiles.append(ut)
                        elif kp == 5:
                            xt_ = s2pool.tile([128, 2 * QS], F16,
                                              name="x_sb", tag="s2")
                            nc.vector.tensor_add(xt_, t_tiles[2],
                                                 t_tiles[1 + 2])
                            t_tiles.append(xt_)
                    if kp >= 2:
                        pull(filler)
                    if kp == 4:
                        emit_av(0)
                        emit_av(1)
                    elif kp >= 5:
                        emit_av(kp - 3)

                def tail(mid=None):
                    nonlocal dn_ps
                    t3 = s2pool.tile([128, 2 * QS], F16, name="t3_sb",
                                     tag="s2")
                    nc.gpsimd.tensor_add(t3, p_tiles[6], p_tiles[7])
                    yt = s2pool.tile([128, 2 * QS], F16, name="y_sb",
                                     tag="s2")
                    nc.vector.tensor_add(yt, t_tiles[4], t3)
                    emit_av(NPAIR - 3)
                    emit_av(NPAIR - 2)
                    emit_av(NPAIR - 1)
                    dn_ps = ps.tile([128, QS], F32, name="dn_ps", tag="acc",
                                    bufs=2)
                    dn_mm(yt, first=True, last=True)
                    if mid is not None:
                        mid()
                    recip = rpool.tile([128, QS], F32, name="recip",
                                       tag="rc")
                    nc.vector.reciprocal_approx_fast(recip, dn_ps)
                    nc.vector.tensor_mul(
                        avt_b[:, h, qs * QS:(qs + 1) * QS], av_ps, recip)
                    pull(filler)
                    pull(filler)
                return tail

            def outproj_gen(b, qs, avt_b, split):
                for tloc in range(QS // 128):
                    tch = qs * (QS // 128) + tloc
                    out_sb = opool.tile([128, D], F16, name="out_sb",
                                        tag="ot")
                    for dsp in range(4):
                        ops = ps.tile([128, 512], F32, name="ops", tag="pj",
                                      bufs=2)
                        for h in range(HP):
                            nc.tensor.matmul(
                                ops, avt_b[:, h, tch * 128:(tch + 1) * 128],
                                wo_sb[:, h, dsp * 512:(dsp + 1) * 512],
                                start=(h == 0), stop=(h == HP - 1))
                        if split[dsp] == "v":
                            nc.vector.tensor_copy(
                                out_sb[:, dsp * 512:(dsp + 1) * 512], ops)
                        elif split[dsp] == "s":
                            nc.scalar.copy(
                                out_sb[:, dsp * 512:(dsp + 1) * 512], ops)
                        else:   # "2": halves on both engines in parallel
                            nc.vector.tensor_copy(
                                out_sb[:, dsp * 512:dsp * 512 + 256],
                                ops[:, 0:256])
                            nc.scalar.copy(
                                out_sb[:, dsp * 512 + 256:(dsp + 1) * 512],
                                ops[:, 256:512])
                        if dsp == 3:
                            nc.sync.dma_start(
                                out=out[b * S + tch * 128:
                                        b * S + (tch + 1) * 128, :],
                                in_=out_sb)
                        yield

            carry = None          # half-consumed outproj of (b-1, qs=3)
            for b in range(B):
                qt_b = qkpool.tile([128, HP, S], F16, name="qt_b", tag="qt")
                kt_b = qkpool.tile([128, HP, S], F16, name="kt_b", tag="kt")
                v_b = bpool.tile([128, NKC, DC], F16, name="v_b", tag="v")
                avt_b = avpool.tile([128, HP, S], F16, name="avt_b",
                                    tag="avt")

                proj_pass(b, wq_sb, bq_sb, qt_b)
                proj_pass(b, wk_sb, bk_sb, kt_b)
                v_pass(b, v_b)

                if b == 0:
                    for i in range(4):
                        nc.sync.dma_start(
                            out=wo_sb[:, :, 512 * i:512 * (i + 1)],
                            in_=wo_r[:, :, 512 * i:512 * (i + 1)])
                    nc.sync.dma_start(out=ones_sb, in_=ones[:, :])

                warm = [warm_pair(0, 0, 0, qt_b, kt_b),
                        warm_pair(0, 0, 1, qt_b, kt_b)]
                for qs in range(NQS):
                    if qs == 0:
                        filler = carry       # leftovers (may be exhausted)
                    else:
                        filler = outproj_gen(b, qs - 1, avt_b, "vvvv")
                    tail = attn_span(qs, 0, qt_b, kt_b, v_b, avt_b, filler,
                                     warm)
                    warm = [warm_pair(qs, 1, 0, qt_b, kt_b)]
                    tail(mid=lambda: warm.append(
                        warm_pair(qs, 1, 1, qt_b, kt_b)))
                    if qs == 0 and b + 1 < B:
                        for sp in range(NSPAN):
                            x_dma(b + 1, sp)
                    tail = attn_span(qs, 1, qt_b, kt_b, v_b, avt_b, filler,
                                     warm)
                    if qs + 1 < NQS:
                        warm = [warm_pair(qs + 1, 0, 0, qt_b, kt_b)]
                        tail(mid=lambda q=qs: warm.append(
                            warm_pair(q + 1, 0, 1, qt_b, kt_b)))
                    else:
                        warm = None
                        tail()
                    if filler is not None:
                        for _ in filler:     # drain any leftovers
                            pass
                carry = outproj_gen(b, NQS - 1, avt_b,
                                    "vvvv" if b + 1 < B else "2222")

            if carry is not None:            # last batch's final span:
                for _ in carry:                  # drain with copies split
                    pass                         # across ACT+DVE (both idle)
    nc.compile()
    _BUILT = nc
    return nc


def _install_trace_hooks():
    import types
    try:
        import antenv.axon_hooks  # noqa: F401
        return True
    except ImportError:
        pass
    try:
        from trn_agent_boot.trn_boot import _ntff_profile_via_ctypes
        hook = _ntff_profile_via_ctypes('/opt/axon/libaxon_pjrt.so')
        if hook is None:
            return False
        m = types.ModuleType('antenv.axon_hooks')
        m.get_axon_ntff_profile_hook = lambda: hook
        sys.modules['antenv.axon_hooks'] = m
        from concourse import bass_utils
        bass_utils.upload_artifacts = lambda tmpdir: "local://" + tmpdir
        return True
    except Exception:
        return False


def kernel(x, wq, bq, wk, bk, wv, bv, wo, bo):
    global LAST_EXEC_NS
    from concourse.bass_utils import run_bass_kernel_spmd

    x = np.asarray(x, dtype=np.float32)
    wq = np.asarray(wq, dtype=np.float32)
    bq = np.asarray(bq, dtype=np.float32)
    wk = np.asarray(wk, dtype=np.float32)
    bk = np.asarray(bk, dtype=np.float32)
    wv = np.asarray(wv, dtype=np.float32)
    bv = np.asarray(bv, dtype=np.float32)
    wo = np.asarray(wo, dtype=np.float32)
    bo = np.asarray(bo, dtype=np.float32)

    xt = np.ascontiguousarray(x.reshape(TOK, D).T).astype(np.float16)
    ones = np.ones((128, 128), dtype=np.float16)
    in_maps = []
    for i in range(NCORES):
        sl = slice(i * DC, (i + 1) * DC)
        in_maps.append({
            "xt": xt,
            "wq": np.ascontiguousarray(wq[:, sl]).astype(np.float16),
            "wk": np.ascontiguousarray(wk[:, sl]).astype(np.float16),
            "wv": np.ascontiguousarray(wv[:, sl]).astype(np.float16),
            "wo": np.ascontiguousarray(wo[sl, :]).astype(np.float16),
            "bq2": np.ascontiguousarray(bq[sl].reshape(HP, HD).T),
            "bk2": np.ascontiguousarray(bk[sl].reshape(HP, HD).T),
            "ones": ones,
        })

    trace = bool(os.environ.get("KERNEL_TRACE"))
    if trace:
        trace = _install_trace_hooks()

    nc = _build()
    res = run_bass_kernel_spmd(nc, in_maps, list(range(NCORES)), trace=trace)
    LAST_EXEC_NS = res.exec_time_ns

    total = np.zeros((TOK, D), dtype=np.float32)
    for r in res.results:
        total += r["out"]
    # V-bias folds into a constant row: softmax rows sum to 1, so
    # attention(V + 1*bv^T) = attention(V) + 1*bv^T, and (bv @ wo) adds to bo.
    total += bo + bv @ wo
    return total.reshape(B, S, D)


# revision 12
# speedup vs baseline: 1.0233x; 1.0233x over previous
"""Multi-head attention (B=4, S=2048, D=2048, H=16) on 8 trn2 NeuronCores.

Sharding: tensor-parallel over heads — 2 heads per core. Each core computes
its heads' Q/K/V projections, full attention for those heads, and a partial
output projection (its 256 rows of wo). The host sums the 8 partial outputs.

v2 (fp16 datapath, gap-free PE schedule):
  - every matmul operand is fp16 (x, weights, Q/K/V, exp tiles, attention
    output); PSUM accumulation stays fp32. fp16 keeps 10 mantissa bits
    (same relative precision as tf32) while halving DMA/SBUF/LDWEIGHTS.
  - softmax denominator: exp tiles are partial-summed on the DVE (fp16
    tensor_tensor, depth-2 tree -> two [128, 2*QS] accumulators per head/
    span), then 4 ones-matmuls partition-reduce into PSUM. Removes the
    512 N=512 denominator matmuls (~119us of PE) for ~8us of DVE per span.
  - the ACT exp stream (1106ns per pair-tile) is slightly slower than the
    scores+AV matmuls (864ns), so the PE would drift into exp-waits. The
    out-projection of span qs-1 is therefore emitted as 16 small groups
    interleaved INTO span qs's pair loops (and the last span's into the
    next batch's Q projection pass) — the PE always has dependency-free
    work queued and batch boundaries have no drain/pstate-ramp gap.
  - PSUM->SBUF output copies are split 4/12 between ACT and DVE to keep
    both helper engines below the PE rate.
  - b=0 startup: wq chunk-DMAs interleave with the first x span, then all
    x spans stream before wk/wv (Q pass is DMA-paced, K/V passes are not);
    first matmul at ~13us.
"""
import os
import sys

sys.path.insert(0, "/opt/trn_rl_repo")
import numpy as np

B, S, D, H = 4, 2048, 2048, 16
HD = 128
NCORES = 8
HP = H // NCORES          # heads per core = 2
DC = HP * HD              # per-core slice of D = 256
TOK = B * S               # 8192
SCALE = HD ** -0.5
NDC = D // 128            # 16 contraction chunks for the projections
SPAN = 256                # token span per projection step
NSPAN = S // SPAN         # 8 spans per batch
QS = 512                  # query span in attention
NQS = S // QS             # 4
NKC = S // 128            # 16 key chunks
NPAIR = NKC // 2          # 8 key-chunk pairs

LAST_EXEC_NS = None
_BUILT = None


def _build():
    global _BUILT
    if _BUILT is not None:
        return _BUILT
    import concourse.tile as tile
    from concourse import bacc, mybir

    F16 = mybir.dt.float16
    F32 = mybir.dt.float32
    Exp = mybir.ActivationFunctionType.Exp
    Ident = mybir.ActivationFunctionType.Identity

    nc = bacc.Bacc("TRN2", target_bir_lowering=False, debug=False)
    xt = nc.dram_tensor("xt", [D, TOK], F16, kind="ExternalInput")
    wq = nc.dram_tensor("wq", [D, DC], F16, kind="ExternalInput")
    wk = nc.dram_tensor("wk", [D, DC], F16, kind="ExternalInput")
    wv = nc.dram_tensor("wv", [D, DC], F16, kind="ExternalInput")
    wo = nc.dram_tensor("wo", [DC, D], F16, kind="ExternalInput")
    bq2 = nc.dram_tensor("bq2", [HD, HP], F32, kind="ExternalInput")
    bk2 = nc.dram_tensor("bk2", [HD, HP], F32, kind="ExternalInput")
    ones = nc.dram_tensor("ones", [128, 128], F16, kind="ExternalInput")
    out = nc.dram_tensor("out", [TOK, D], F16, kind="ExternalOutput")

    with tile.TileContext(nc) as tc:
        with tc.tile_pool(name="const", bufs=1) as cpool, \
             tc.tile_pool(name="xp", bufs=8) as xpool, \
             tc.tile_pool(name="bt", bufs=1) as bpool, \
             tc.tile_pool(name="qk", bufs=2) as qkpool, \
             tc.tile_pool(name="avp", bufs=2) as avpool, \
             tc.tile_pool(name="pp", bufs=5) as ppool, \
             tc.tile_pool(name="s2", bufs=8) as s2pool, \
             tc.tile_pool(name="rc", bufs=2) as rpool, \
             tc.tile_pool(name="ot", bufs=2) as opool, \
             tc.tile_pool(name="ps", bufs=1, space="PSUM") as ps:

            wq_sb = cpool.tile([128, NDC, DC], F16)
            wk_sb = cpool.tile([128, NDC, DC], F16)
            wv_sb = cpool.tile([128, NDC, DC], F16)
            wo_sb = cpool.tile([128, HP, D], F16)
            ones_sb = cpool.tile([128, 128], F16)
            bq_sb = cpool.tile([HD, HP], F32)
            bk_sb = cpool.tile([HD, HP], F32)

            wq_r = wq.rearrange("(c p) n -> p c n", p=128)
            wk_r = wk.rearrange("(c p) n -> p c n", p=128)
            wv_r = wv.rearrange("(c p) n -> p c n", p=128)
            wo_r = wo.rearrange("(c p) n -> p c n", p=128)
            xt_r = xt.rearrange("(c p) t -> p c t", p=128)

            nc.sync.dma_start(out=bq_sb, in_=bq2[:, :])
            nc.sync.dma_start(out=bk_sb, in_=bk2[:, :])

            xts = {}

            def x_dma(b, sp):
                t0 = b * S + sp * SPAN
                xtl = xpool.tile([128, NDC, SPAN], F16, name=f"x{b}_{sp}",
                                 tag="x")
                xts[(b, sp)] = xtl
                nc.sync.dma_start(out=xtl, in_=xt_r[:, :, t0:t0 + SPAN])

            # --- b=0 startup: wq chunks interleave with the first x span;
            # remaining x spans stream before wk/wv (the Q pass is DMA-paced,
            # the K/V passes run much later) ---
            x00 = xpool.tile([128, NDC, SPAN], F16, name="x0_0", tag="x")
            xts[(0, 0)] = x00
            for i in range(4):
                nc.sync.dma_start(out=wq_sb[:, 4 * i:4 * i + 4, :],
                                  in_=wq_r[:, 4 * i:4 * i + 4, :])
                nc.sync.dma_start(out=x00[:, 4 * i:4 * i + 4, :],
                                  in_=xt_r[:, 4 * i:4 * i + 4, 0:SPAN])
            for sp in range(1, NSPAN):
                x_dma(0, sp)
            for i in range(2):
                nc.sync.dma_start(out=wk_sb[:, 8 * i:8 * i + 8, :],
                                  in_=wk_r[:, 8 * i:8 * i + 8, :])
            for i in range(2):
                nc.sync.dma_start(out=wv_sb[:, 8 * i:8 * i + 8, :],
                                  in_=wv_r[:, 8 * i:8 * i + 8, :])

            def pull(filler):
                if filler is not None:
                    next(filler, None)

            def proj_pass(b, w_sb, b_sb, dst):
                for sp in range(NSPAN):
                    xtl = xts[(b, sp)]
                    for h in range(HP):
                        pps = ps.tile([128, SPAN], F32, name="pps", tag="pj",
                                      bufs=2)
                        for c in range(NDC):
                            nc.tensor.matmul(
                                pps, w_sb[:, c, h * HD:(h + 1) * HD],
                                xtl[:, c, :], start=(c == 0),
                                stop=(c == NDC - 1))
                        nc.scalar.activation(
                            dst[:, h, sp * SPAN:(sp + 1) * SPAN], pps, Ident,
                            bias=b_sb[:, h:h + 1])

            def v_pass(b, v_b):
                for sp in range(NSPAN):
                    xtl = xts[(b, sp)]
                    vps = ps.tile([128, 2 * DC], F32, name="vps", tag="pj",
                                  bufs=2)
                    for tch in range(2):
                        for c in range(NDC):
                            nc.tensor.matmul(
                                vps[:, tch * DC:(tch + 1) * DC],
                                xtl[:, c, tch * 128:(tch + 1) * 128],
                                wv_sb[:, c, :], start=(c == 0),
                                stop=(c == NDC - 1))
                    for tch in range(2):
                        nc.scalar.copy(v_b[:, sp * 2 + tch, :],
                                       vps[:, tch * DC:(tch + 1) * DC])

            def warm_pair(qs, h, kp, qt_b, kt_b):
                # scores+exp of an upcoming half-span pair, emitted before
                # the current tail so the ACT exp stream never restarts cold
                q_sl = qt_b[:, h, qs * QS:(qs + 1) * QS]
                s_ps = ps.tile([128, 2 * QS], F32, name="s_ps", tag="s",
                               bufs=2)
                for j in range(2):
                    kc = 2 * kp + j
                    nc.tensor.matmul(
                        s_ps[:, j * QS:(j + 1) * QS],
                        kt_b[:, h, kc * 128:(kc + 1) * 128], q_sl,
                        start=True, stop=True)
                pt = ppool.tile([128, 2 * QS], F16, name="p_sb", tag="p")
                nc.scalar.activation(pt, s_ps, Exp, scale=SCALE)
                return pt

            def attn_span(qs, h, qt_b, kt_b, v_b, avt_b, filler=None,
                          warm=None):
                q_sl = qt_b[:, h, qs * QS:(qs + 1) * QS]
                av_ps = ps.tile([HD, QS], F32, name="av_ps", tag="acc",
                                bufs=2)
                p_tiles = []
                t_tiles = []
                dn_ps = None

                def emit_av(kp):
                    pt = p_tiles[kp]
                    for j in range(2):
                        kc = 2 * kp + j
                        nc.tensor.matmul(
                            av_ps, v_b[:, kc, h * HD:(h + 1) * HD],
                            pt[:, j * QS:(j + 1) * QS], start=(kc == 0),
                            stop=(kc == NKC - 1))

                def dn_mm(src, first=False, last=False):
                    nc.tensor.matmul(dn_ps, ones_sb, src[:, 0:QS],
                                     start=first, stop=False)
                    nc.tensor.matmul(dn_ps, ones_sb, src[:, QS:2 * QS],
                                     start=False, stop=last)

                for kp in range(NPAIR):
                    if warm is not None and kp < len(warm):
                        p_tiles.append(warm[kp])
                    else:
                        s_ps = ps.tile([128, 2 * QS], F32, name="s_ps",
                                       tag="s", bufs=2)
                        for j in range(2):
                            kc = 2 * kp + j
                            nc.tensor.matmul(
                                s_ps[:, j * QS:(j + 1) * QS],
                                kt_b[:, h, kc * 128:(kc + 1) * 128], q_sl,
                                start=True, stop=True)
                        pt = ppool.tile([128, 2 * QS], F16, name="p_sb",
                                        tag="p")
                        nc.scalar.activation(pt, s_ps, Exp, scale=SCALE)
                        p_tiles.append(pt)
                    # pair-sum tree: pair-adds on the idle GPSIMD
                    # (latency-tolerant), combine levels U and X=U+T2 on the
                    # DVE; the last exp pair feeds the ones-matmuls directly
                    # so the normalize chain never waits a slow add
                    if kp in (1, 3, 5):
                        tt = s2pool.tile([128, 2 * QS], F16, name="t_sb",
                                         tag="s2")
                        nc.gpsimd.tensor_add(tt, p_tiles[kp - 1],
                                             p_tiles[kp])
                        t_tiles.append(tt)
                        if kp == 3:
                            ut = s2pool.tile([128, 2 * QS], F16, name="u_sb",
                                             tag="s2")
                            nc.vector.tensor_add(ut, t_tiles[0], t_tiles[1])
                            t_tiles.append(ut)
                        elif kp == 5:
                            xt_ = s2pool.tile([128, 2 * QS], F16,
                                              name="x_sb", tag="s2")
                            nc.vector.tensor_add(xt_, t_tiles[2],
                                                 t_tiles[3])
                            t_tiles.append(xt_)
                    if kp >= 2:
                        pull(filler)
                    if kp == 3:
                        emit_av(0)
                        emit_av(1)
                    elif kp >= 4:
                        emit_av(kp - 2)

                def tail(mid=None):
                    nonlocal dn_ps
                    dn_ps = ps.tile([128, QS], F32, name="dn_ps", tag="acc",
                                    bufs=2)
                    dn_mm(t_tiles[4], first=True)      # X = p0..p5
                    emit_av(NPAIR - 2)
                    dn_mm(p_tiles[6])
                    emit_av(NPAIR - 1)
                    dn_mm(p_tiles[7], last=True)
                    if mid is not None:
                        mid()
                    recip = rpool.tile([128, QS], F32, name="recip",
                                       tag="rc")
                    nc.vector.reciprocal_approx_fast(recip, dn_ps)
                    nc.vector.tensor_mul(
                        avt_b[:, h, qs * QS:(qs + 1) * QS], av_ps, recip)
                    pull(filler)
                    pull(filler)
                return tail

            def outproj_gen(b, qs, avt_b, split):
                for tloc in range(QS // 128):
                    tch = qs * (QS // 128) + tloc
                    out_sb = opool.tile([128, D], F16, name="out_sb",
                                        tag="ot")
                    for dsp in range(4):
                        ops = ps.tile([128, 512], F32, name="ops", tag="pj",
                                      bufs=2)
                        for h in range(HP):
                            nc.tensor.matmul(
                                ops, avt_b[:, h, tch * 128:(tch + 1) * 128],
                                wo_sb[:, h, dsp * 512:(dsp + 1) * 512],
                                start=(h == 0), stop=(h == HP - 1))
                        if split[dsp] == "v":
                            nc.vector.tensor_copy(
                                out_sb[:, dsp * 512:(dsp + 1) * 512], ops)
                        elif split[dsp] == "s":
                            nc.scalar.copy(
                                out_sb[:, dsp * 512:(dsp + 1) * 512], ops)
                        else:   # "2": halves on both engines in parallel
                            nc.vector.tensor_copy(
                                out_sb[:, dsp * 512:dsp * 512 + 256],
                                ops[:, 0:256])
                            nc.scalar.copy(
                                out_sb[:, dsp * 512 + 256:(dsp + 1) * 512],
                                ops[:, 256:512])
                        if dsp == 3:
                            nc.sync.dma_start(
                                out=out[b * S + tch * 128:
                                        b * S + (tch + 1) * 128, :],
                                in_=out_sb)
                        yield

            carry = None          # half-consumed outproj of (b-1, qs=3)
            for b in range(B):
                qt_b = qkpool.tile([128, HP, S], F16, name="qt_b", tag="qt")
                kt_b = qkpool.tile([128, HP, S], F16, name="kt_b", tag="kt")
                v_b = bpool.tile([128, NKC, DC], F16, name="v_b", tag="v")
                avt_b = avpool.tile([128, HP, S], F16, name="avt_b",
                                    tag="avt")

                proj_pass(b, wq_sb, bq_sb, qt_b)
                proj_pass(b, wk_sb, bk_sb, kt_b)
                v_pass(b, v_b)

                if b == 0:
                    for i in range(4):
                        nc.sync.dma_start(
                            out=wo_sb[:, :, 512 * i:512 * (i + 1)],
                            in_=wo_r[:, :, 512 * i:512 * (i + 1)])
                    nc.sync.dma_start(out=ones_sb, in_=ones[:, :])

                warm = [warm_pair(0, 0, 0, qt_b, kt_b),
                        warm_pair(0, 0, 1, qt_b, kt_b)]
                for qs in range(NQS):
                    if qs == 0:
                        filler = carry       # leftovers (may be exhausted)
                    else:
                        filler = outproj_gen(b, qs - 1, avt_b, "vvvv")
                    tail = attn_span(qs, 0, qt_b, kt_b, v_b, avt_b, filler,
                                     warm)
                    warm = [warm_pair(qs, 1, 0, qt_b, kt_b)]
                    tail(mid=lambda: warm.append(
                        warm_pair(qs, 1, 1, qt_b, kt_b)))
                    if qs == 0 and b + 1 < B:
                        for sp in range(NSPAN):
                            x_dma(b + 1, sp)
                    tail = attn_span(qs, 1, qt_b, kt_b, v_b, avt_b, filler,
                                     warm)
                    if qs + 1 < NQS:
                        warm = [warm_pair(qs + 1, 0, 0, qt_b, kt_b)]
                        tail(mid=lambda q=qs: warm.append(
                            warm_pair(q + 1, 0, 1, qt_b, kt_b)))
                    else:
                        warm = None
                        tail()
                    if filler is not None:
                        for _ in filler:     # drain any leftovers
                            pass
                carry = outproj_gen(b, NQS - 1, avt_b,
                                    "vvvv" if b + 1 < B else "2222")

            if carry is not None:            # last batch's final span:
                for _ in carry:                  # drain with copies split
                    pass                         # across ACT+DVE (both idle)
    nc.compile()
    _BUILT = nc
    return nc


def _install_trace_hooks():
    import types
    try:
        import antenv.axon_hooks  # noqa: F401
        return True
    except ImportError:
        pass
    try:
        from trn_agent_boot.trn_boot import _ntff_profile_via_ctypes
        hook = _ntff_profile_via_ctypes('/opt/axon/libaxon_pjrt.so')
        if hook is None:
            return False
        m = types.ModuleType('antenv.axon_hooks')
        m.get_axon_ntff_profile_hook = lambda: hook
        sys.modules['antenv.axon_hooks'] = m
        from concourse import bass_utils
        bass_utils.upload_artifacts = lambda tmpdir: "local://" + tmpdir
        return True
    except Exception:
        return False


def kernel(x, wq, bq, wk, bk, wv, bv, wo, bo):
    global LAST_EXEC_NS
    from concourse.bass_utils import run_bass_kernel_spmd

    x = np.asarray(x, dtype=np.float32)
    wq = np.asarray(wq, dtype=np.float32)
    bq = np.asarray(bq, dtype=np.float32)
    wk = np.asarray(wk, dtype=np.float32)
    bk = np.asarray(bk, dtype=np.float32)
    wv = np.asarray(wv, dtype=np.float32)
    bv = np.asarray(bv, dtype=np.float32)
    wo = np.asarray(wo, dtype=np.float32)
    bo = np.asarray(bo, dtype=np.float32)

    xt = np.ascontiguousarray(x.reshape(TOK, D).T).astype(np.float16)
    ones = np.ones((128, 128), dtype=np.float16)
    in_maps = []
    for i in range(NCORES):
        sl = slice(i * DC, (i + 1) * DC)
        in_maps.append({
            "xt": xt,
            "wq": np.ascontiguousarray(wq[:, sl]).astype(np.float16),
            "wk": np.ascontiguousarray(wk[:, sl]).astype(np.float16),
            "wv": np.ascontiguousarray(wv[:, sl]).astype(np.float16),
            "wo": np.ascontiguousarray(wo[sl, :]).astype(np.float16),
            "bq2": np.ascontiguousarray(bq[sl].reshape(HP, HD).T),
            "bk2": np.ascontiguousarray(bk[sl].reshape(HP, HD).T),
            "ones": ones,
        })

    trace = bool(os.environ.get("KERNEL_TRACE"))
    if trace:
        trace = _install_trace_hooks()

    nc = _build()
    res = run_bass_kernel_spmd(nc, in_maps, list(range(NCORES)), trace=trace)
    LAST_EXEC_NS = res.exec_time_ns

    total = np.zeros((TOK, D), dtype=np.float32)
    for r in res.results:
        total += r["out"]
    # V-bias folds into a constant row: softmax rows sum to 1, so
    # attention(V + 1*bv^T) = attention(V) + 1*bv^T, and (bv @ wo) adds to bo.
    total += bo + bv @ wo
    return total.reshape(B, S, D)


# revision 14
# speedup vs baseline: 1.0416x; 1.0179x over previous
"""Multi-head attention (B=4, S=2048, D=2048, H=16) on 8 trn2 NeuronCores.

Sharding: tensor-parallel over heads — 2 heads per core. Each core computes
its heads' Q/K/V projections, full attention for those heads, and a partial
output projection (its 256 rows of wo). The host sums the 8 partial outputs.

v2 (fp16 datapath, gap-free PE schedule):
  - every matmul operand is fp16 (x, weights, Q/K/V, exp tiles, attention
    output); PSUM accumulation stays fp32. fp16 keeps 10 mantissa bits
    (same relative precision as tf32) while halving DMA/SBUF/LDWEIGHTS.
  - softmax denominator: exp tiles are partial-summed on the DVE (fp16
    tensor_tensor, depth-2 tree -> two [128, 2*QS] accumulators per head/
    span), then 4 ones-matmuls partition-reduce into PSUM. Removes the
    512 N=512 denominator matmuls (~119us of PE) for ~8us of DVE per span.
  - the ACT exp stream (1106ns per pair-tile) is slightly slower than the
    scores+AV matmuls (864ns), so the PE would drift into exp-waits. The
    out-projection of span qs-1 is therefore emitted as 16 small groups
    interleaved INTO span qs's pair loops (and the last span's into the
    next batch's Q projection pass) — the PE always has dependency-free
    work queued and batch boundaries have no drain/pstate-ramp gap.
  - PSUM->SBUF output copies are split 4/12 between ACT and DVE to keep
    both helper engines below the PE rate.
  - b=0 startup: wq chunk-DMAs interleave with the first x span, then all
    x spans stream before wk/wv (Q pass is DMA-paced, K/V passes are not);
    first matmul at ~13us.
"""
import os
import sys

sys.path.insert(0, "/opt/trn_rl_repo")
import numpy as np

B, S, D, H = 4, 2048, 2048, 16
HD = 128
NCORES = 8
HP = H // NCORES          # heads per core = 2
DC = HP * HD              # per-core slice of D = 256
TOK = B * S               # 8192
SCALE = HD ** -0.5
NDC = D // 128            # 16 contraction chunks for the projections
SPAN = 256                # token span per projection step
NSPAN = S // SPAN         # 8 spans per batch
QS = 512                  # query span in attention
NQS = S // QS             # 4
NKC = S // 128            # 16 key chunks
NPAIR = NKC // 2          # 8 key-chunk pairs

LAST_EXEC_NS = None
_BUILT = None


def _build():
    global _BUILT
    if _BUILT is not None:
        return _BUILT
    import concourse.tile as tile
    from concourse import bacc, mybir

    F16 = mybir.dt.float16
    F32 = mybir.dt.float32
    Exp = mybir.ActivationFunctionType.Exp
    Ident = mybir.ActivationFunctionType.Identity

    nc = bacc.Bacc("TRN2", target_bir_lowering=False, debug=False)
    xt = nc.dram_tensor("xt", [D, TOK], F16, kind="ExternalInput")
    wq = nc.dram_tensor("wq", [D, DC], F16, kind="ExternalInput")
    wk = nc.dram_tensor("wk", [D, DC], F16, kind="ExternalInput")
    wv = nc.dram_tensor("wv", [D, DC], F16, kind="ExternalInput")
    wo = nc.dram_tensor("wo", [DC, D], F16, kind="ExternalInput")
    bq2 = nc.dram_tensor("bq2", [HD, HP], F32, kind="ExternalInput")
    bk2 = nc.dram_tensor("bk2", [HD, HP], F32, kind="ExternalInput")
    ones = nc.dram_tensor("ones", [128, 128], F16, kind="ExternalInput")
    out = nc.dram_tensor("out", [TOK, D], F16, kind="ExternalOutput")

    with tile.TileContext(nc) as tc:
        with tc.tile_pool(name="const", bufs=1) as cpool, \
             tc.tile_pool(name="xp", bufs=8) as xpool, \
             tc.tile_pool(name="bt", bufs=1) as bpool, \
             tc.tile_pool(name="qk", bufs=2) as qkpool, \
             tc.tile_pool(name="avp", bufs=2) as avpool, \
             tc.tile_pool(name="pp", bufs=5) as ppool, \
             tc.tile_pool(name="s2", bufs=8) as s2pool, \
             tc.tile_pool(name="rc", bufs=2) as rpool, \
             tc.tile_pool(name="ot", bufs=2) as opool, \
             tc.tile_pool(name="ps", bufs=1, space="PSUM") as ps:

            wq_sb = cpool.tile([128, NDC, DC], F16)
            wk_sb = cpool.tile([128, NDC, DC], F16)
            wv_sb = cpool.tile([128, NDC, DC], F16)
            wo_sb = cpool.tile([128, HP, D], F16)
            ones_sb = cpool.tile([128, 128], F16)
            bq_sb = cpool.tile([HD, HP], F32)
            bk_sb = cpool.tile([HD, HP], F32)

            wq_r = wq.rearrange("(c p) n -> p c n", p=128)
            wk_r = wk.rearrange("(c p) n -> p c n", p=128)
            wv_r = wv.rearrange("(c p) n -> p c n", p=128)
            wo_r = wo.rearrange("(c p) n -> p c n", p=128)
            xt_r = xt.rearrange("(c p) t -> p c t", p=128)

            nc.sync.dma_start(out=bq_sb, in_=bq2[:, :])
            nc.sync.dma_start(out=bk_sb, in_=bk2[:, :])

            xts = {}

            def x_dma(b, sp):
                t0 = b * S + sp * SPAN
                xtl = xpool.tile([128, NDC, SPAN], F16, name=f"x{b}_{sp}",
                                 tag="x")
                xts[(b, sp)] = xtl
                nc.sync.dma_start(out=xtl, in_=xt_r[:, :, t0:t0 + SPAN])

            # --- b=0 startup: wq chunks interleave with the first x span;
            # remaining x spans stream before wk/wv (the Q pass is DMA-paced,
            # the K/V passes run much later) ---
            x00 = xpool.tile([128, NDC, SPAN], F16, name="x0_0", tag="x")
            xts[(0, 0)] = x00
            for i in range(4):
                nc.sync.dma_start(out=wq_sb[:, 4 * i:4 * i + 4, :],
                                  in_=wq_r[:, 4 * i:4 * i + 4, :])
                nc.sync.dma_start(out=x00[:, 4 * i:4 * i + 4, :],
                                  in_=xt_r[:, 4 * i:4 * i + 4, 0:SPAN])
            for sp in range(1, 4):
                x_dma(0, sp)
            for i in range(2):
                nc.sync.dma_start(out=wk_sb[:, 8 * i:8 * i + 8, :],
                                  in_=wk_r[:, 8 * i:8 * i + 8, :])
            for sp in range(4, NSPAN):
                x_dma(0, sp)
            for i in range(2):
                nc.sync.dma_start(out=wv_sb[:, 8 * i:8 * i + 8, :],
                                  in_=wv_r[:, 8 * i:8 * i + 8, :])

            def pull(filler):
                if filler is not None:
                    next(filler, None)

            def proj_pass(b, w_sb, b_sb, dst, spans=None):
                for sp in (spans if spans is not None else range(NSPAN)):
                    xtl = xts[(b, sp)]
                    for h in range(HP):
                        pps = ps.tile([128, SPAN], F32, name="pps", tag="pj",
                                      bufs=2)
                        for c in range(NDC):
                            nc.tensor.matmul(
                                pps, w_sb[:, c, h * HD:(h + 1) * HD],
                                xtl[:, c, :], start=(c == 0),
                                stop=(c == NDC - 1))
                        nc.scalar.activation(
                            dst[:, h, sp * SPAN:(sp + 1) * SPAN], pps, Ident,
                            bias=b_sb[:, h:h + 1])

            def v_pass(b, v_b):
                for sp in range(NSPAN):
                    xtl = xts[(b, sp)]
                    vps = ps.tile([128, 2 * DC], F32, name="vps", tag="pj",
                                  bufs=2)
                    for tch in range(2):
                        for c in range(NDC):
                            nc.tensor.matmul(
                                vps[:, tch * DC:(tch + 1) * DC],
                                xtl[:, c, tch * 128:(tch + 1) * 128],
                                wv_sb[:, c, :], start=(c == 0),
                                stop=(c == NDC - 1))
                    for tch in range(2):
                        nc.scalar.copy(v_b[:, sp * 2 + tch, :],
                                       vps[:, tch * DC:(tch + 1) * DC])

            def warm_pair(qs, h, kp, qt_b, kt_b):
                # scores+exp of an upcoming half-span pair, emitted before
                # the current tail so the ACT exp stream never restarts cold
                q_sl = qt_b[:, h, qs * QS:(qs + 1) * QS]
                s_ps = ps.tile([128, 2 * QS], F32, name="s_ps", tag="s",
                               bufs=2)
                for j in range(2):
                    kc = 2 * kp + j
                    nc.tensor.matmul(
                        s_ps[:, j * QS:(j + 1) * QS],
                        kt_b[:, h, kc * 128:(kc + 1) * 128], q_sl,
                        start=True, stop=True)
                pt = ppool.tile([128, 2 * QS], F16, name="p_sb", tag="p")
                nc.scalar.activation(pt, s_ps, Exp, scale=SCALE)
                return pt

            def attn_span(qs, h, qt_b, kt_b, v_b, avt_b, filler=None,
                          warm=None):
                q_sl = qt_b[:, h, qs * QS:(qs + 1) * QS]
                av_ps = ps.tile([HD, QS], F32, name="av_ps", tag="acc",
                                bufs=2)
                p_tiles = []
                t_tiles = []
                dn_ps = None

                def emit_av(kp):
                    pt = p_tiles[kp]
                    for j in range(2):
                        kc = 2 * kp + j
                        nc.tensor.matmul(
                            av_ps, v_b[:, kc, h * HD:(h + 1) * HD],
                            pt[:, j * QS:(j + 1) * QS], start=(kc == 0),
                            stop=(kc == NKC - 1))

                def dn_mm(src, first=False, last=False):
                    nc.tensor.matmul(dn_ps, ones_sb, src[:, 0:QS],
                                     start=first, stop=False)
                    nc.tensor.matmul(dn_ps, ones_sb, src[:, QS:2 * QS],
                                     start=False, stop=last)

                for kp in range(NPAIR):
                    if warm is not None and kp < len(warm):
                        p_tiles.append(warm[kp])
                    else:
                        s_ps = ps.tile([128, 2 * QS], F32, name="s_ps",
                                       tag="s", bufs=2)
                        for j in range(2):
                            kc = 2 * kp + j
                            nc.tensor.matmul(
                                s_ps[:, j * QS:(j + 1) * QS],
                                kt_b[:, h, kc * 128:(kc + 1) * 128], q_sl,
                                start=True, stop=True)
                        pt = ppool.tile([128, 2 * QS], F16, name="p_sb",
                                        tag="p")
                        nc.scalar.activation(pt, s_ps, Exp, scale=SCALE)
                        p_tiles.append(pt)
                    # pair-sum tree: first two pairs on the idle GPSIMD
                    # (latency-tolerant), then U=T0+T1 and T2 on the DVE;
                    # the last exp pair feeds the ones-matmuls directly
                    if kp in (1, 3):
                        tt = s2pool.tile([128, 2 * QS], F16, name="t_sb",
                                         tag="s2")
                        nc.gpsimd.tensor_add(tt, p_tiles[kp - 1],
                                             p_tiles[kp])
                        t_tiles.append(tt)
                        if kp == 3:
                            ut = s2pool.tile([128, 2 * QS], F16, name="u_sb",
                                             tag="s2")
                            nc.vector.tensor_add(ut, t_tiles[0], t_tiles[1])
                            t_tiles.append(ut)
                    elif kp == 5:
                        tt = s2pool.tile([128, 2 * QS], F16, name="t2_sb",
                                         tag="s2")
                        nc.vector.tensor_add(tt, p_tiles[4], p_tiles[5])
                        t_tiles.append(tt)
                    if kp >= 2:
                        pull(filler)
                    if kp == 3:
                        emit_av(0)
                        emit_av(1)
                    elif kp >= 4:
                        emit_av(kp - 2)
                    if kp == 6:
                        # partition-reduce inside the ACT-paced loop where
                        # the PE has idle slots
                        dn_ps = ps.tile([128, QS], F32, name="dn_ps",
                                        tag="acc", bufs=2)
                        dn_mm(t_tiles[2], first=True)      # U = p0..p3
                    elif kp == 7:
                        dn_mm(t_tiles[3])                  # T2 = p4+p5

                def tail(mid=None):
                    emit_av(NPAIR - 2)
                    dn_mm(p_tiles[6])
                    emit_av(NPAIR - 1)
                    dn_mm(p_tiles[7], last=True)
                    if mid is not None:
                        mid()
                    recip = rpool.tile([128, QS], F32, name="recip",
                                       tag="rc")
                    nc.vector.reciprocal_approx_fast(recip, dn_ps)
                    nc.vector.tensor_mul(
                        avt_b[:, h, qs * QS:(qs + 1) * QS], av_ps, recip)
                    pull(filler)
                    pull(filler)
                return tail

            def outproj_gen(b, qs, avt_b, split):
                for tloc in range(QS // 128):
                    tch = qs * (QS // 128) + tloc
                    out_sb = opool.tile([128, D], F16, name="out_sb",
                                        tag="ot")
                    for dsp in range(4):
                        ops = ps.tile([128, 512], F32, name="ops", tag="pj",
                                      bufs=2)
                        for h in range(HP):
                            nc.tensor.matmul(
                                ops, avt_b[:, h, tch * 128:(tch + 1) * 128],
                                wo_sb[:, h, dsp * 512:(dsp + 1) * 512],
                                start=(h == 0), stop=(h == HP - 1))
                        if split[dsp] == "v":
                            nc.vector.tensor_copy(
                                out_sb[:, dsp * 512:(dsp + 1) * 512], ops)
                        elif split[dsp] == "s":
                            nc.scalar.copy(
                                out_sb[:, dsp * 512:(dsp + 1) * 512], ops)
                        else:   # "2": halves on both engines in parallel
                            nc.vector.tensor_copy(
                                out_sb[:, dsp * 512:dsp * 512 + 256],
                                ops[:, 0:256])
                            nc.scalar.copy(
                                out_sb[:, dsp * 512 + 256:(dsp + 1) * 512],
                                ops[:, 256:512])
                        if dsp == 3:
                            nc.sync.dma_start(
                                out=out[b * S + tch * 128:
                                        b * S + (tch + 1) * 128, :],
                                in_=out_sb)
                        yield

            carry = None          # half-consumed outproj of (b-1, qs=3)
            for b in range(B):
                qt_b = qkpool.tile([128, HP, S], F16, name="qt_b", tag="qt")
                kt_b = qkpool.tile([128, HP, S], F16, name="kt_b", tag="kt")
                v_b = bpool.tile([128, NKC, DC], F16, name="v_b", tag="v")
                avt_b = avpool.tile([128, HP, S], F16, name="avt_b",
                                    tag="avt")

                if b == 0:
                    # first batch is DMA-paced: alternate Q/K half-passes so
                    # the PE never outruns the x-span stream
                    proj_pass(b, wq_sb, bq_sb, qt_b, spans=range(0, 4))
                    proj_pass(b, wk_sb, bk_sb, kt_b, spans=range(0, 4))
                    proj_pass(b, wq_sb, bq_sb, qt_b, spans=range(4, 8))
                    proj_pass(b, wk_sb, bk_sb, kt_b, spans=range(4, 8))
                else:
                    proj_pass(b, wq_sb, bq_sb, qt_b)
                    proj_pass(b, wk_sb, bk_sb, kt_b)
                v_pass(b, v_b)

                if b == 0:
                    for i in range(4):
                        nc.sync.dma_start(
                            out=wo_sb[:, :, 512 * i:512 * (i + 1)],
                            in_=wo_r[:, :, 512 * i:512 * (i + 1)])
                    nc.sync.dma_start(out=ones_sb, in_=ones[:, :])

                warm = [warm_pair(0, 0, 0, qt_b, kt_b),
                        warm_pair(0, 0, 1, qt_b, kt_b)]
                for qs in range(NQS):
                    if qs == 0:
                        filler = carry       # leftovers (may be exhausted)
                    else:
                        filler = outproj_gen(b, qs - 1, avt_b, "vvvv")
                    tail = attn_span(qs, 0, qt_b, kt_b, v_b, avt_b, filler,
                                     warm)
                    warm = [warm_pair(qs, 1, 0, qt_b, kt_b)]
                    tail(mid=lambda: warm.append(
                        warm_pair(qs, 1, 1, qt_b, kt_b)))
                    if qs == 0 and b + 1 < B:
                        for sp in range(NSPAN):
                            x_dma(b + 1, sp)
                    tail = attn_span(qs, 1, qt_b, kt_b, v_b, avt_b, filler,
                                     warm)
                    if qs + 1 < NQS:
                        warm = [warm_pair(qs + 1, 0, 0, qt_b, kt_b)]
                        tail(mid=lambda q=qs: warm.append(
                            warm_pair(q + 1, 0, 1, qt_b, kt_b)))
                    else:
                        warm = None
                        tail()
                    if filler is not None:
                        for _ in filler:     # drain any leftovers
                            pass
                carry = outproj_gen(b, NQS - 1, avt_b,
                                    "vvvv" if b + 1 < B else "2222")

            if carry is not None:            # last batch's final span:
                for _ in carry:                  # drain with copies split
                    pass                         # across ACT+DVE (both idle)
    nc.compile()
    _BUILT = nc
    return nc


def _install_trace_hooks():
    import types
    try:
        import antenv.axon_hooks  # noqa: F401
        return True
    except ImportError:
        pass
    try:
        from trn_agent_boot.trn_boot import _ntff_profile_via_ctypes
        hook = _ntff_profile_via_ctypes('/opt/axon/libaxon_pjrt.so')
        if hook is None:
            return False
        m = types.ModuleType('antenv.axon_hooks')
        m.get_axon_ntff_profile_hook = lambda: hook
        sys.modules['antenv.axon_hooks'] = m
        from concourse import bass_utils
        bass_utils.upload_artifacts = lambda tmpdir: "local://" + tmpdir
        return True
    except Exception:
        return False


def kernel(x, wq, bq, wk, bk, wv, bv, wo, bo):
    global LAST_EXEC_NS
    from concourse.bass_utils import run_bass_kernel_spmd

    x = np.asarray(x, dtype=np.float32)
    wq = np.asarray(wq, dtype=np.float32)
    bq = np.asarray(bq, dtype=np.float32)
    wk = np.asarray(wk, dtype=np.float32)
    bk = np.asarray(bk, dtype=np.float32)
    wv = np.asarray(wv, dtype=np.float32)
    bv = np.asarray(bv, dtype=np.float32)
    wo = np.asarray(wo, dtype=np.float32)
    bo = np.asarray(bo, dtype=np.float32)

    xt = np.ascontiguousarray(x.reshape(TOK, D).T).astype(np.float16)
    ones = np.ones((128, 128), dtype=np.float16)
    in_maps = []
    for i in range(NCORES):
        sl = slice(i * DC, (i + 1) * DC)
        in_maps.append({
            "xt": xt,
            "wq": np.ascontiguousarray(wq[:, sl]).astype(np.float16),
            "wk": np.ascontiguousarray(wk[:, sl]).astype(np.float16),
            "wv": np.ascontiguousarray(wv[:, sl]).astype(np.float16),
            "wo": np.ascontiguousarray(wo[sl, :]).astype(np.float16),
            "bq2": np.ascontiguousarray(bq[sl].reshape(HP, HD).T),
            "bk2": np.ascontiguousarray(bk[sl].reshape(HP, HD).T),
            "ones": ones,
        })

    trace = bool(os.environ.get("KERNEL_TRACE"))
    if trace:
        trace = _install_trace_hooks()

    nc = _build()
    res = run_bass_kernel_spmd(nc, in_maps, list(range(NCORES)), trace=trace)
    LAST_EXEC_NS = res.exec_time_ns

    total = np.zeros((TOK, D), dtype=np.float32)
    for r in res.results:
        total += r["out"]
    # V-bias folds into a constant row: softmax rows sum to 1, so
    # attention(V + 1*bv^T) = attention(V) + 1*bv^T, and (bv @ wo) adds to bo.
    total += bo + bv @ wo
    return total.reshape(B, S, D)
